# revision 28
# baseline (speedup 1.0000x reference)
"""BitNet attention (GQA + RoPE) on 8 Trainium2 NeuronCores.

Tensor-parallel over heads: core c owns q-heads [4c, 4c+4), kv-head c.
Each core computes q/k/v projections (ternary BitNet weights), RoPE,
attention for its heads, and a row-parallel partial of the Wo
projection; the host sums the 8 fp16 partials in fp32.

Schedule/dtype strategy (v7, from trace analysis):
  - ALL matmuls fp16: f32r runs at half PE clock on HW; ternary weights
    are exact in fp16, activations lose ~0.05%.
  - All DMA traffic fp16; wv is pre-scaled by s_v*s_o on the host.
  - The attention mask enters as the EXP activation's per-partition
    bias (exp(s*qk + mask_k)), so V needs no mask scaling and is built
    with DMA transposes (no PE transposes, no PSUM scratch).
  - One PSUM layout for the whole kernel (pssc 4 banks / psav 2 /
    pso 2): phase-1 projections accumulate into pssc tiles, RoPE
    rotations into pso tiles, so no bank-reuse stall or PE-clock reset
    (HAM drops the PE to 1.2 GHz after ~3.4us of idle).
  - Phase 1 pipelines kv(b)+q(b) per 512-token block; block 3's q
    projection is deferred into phase 2's per-chunk slack slots.
  - Phase 2 is ScalarE-bound (exp of 16.8M scores, ~1.1us per
    [128,1024] ACTIVATE). Per chunk the PE does one row-tiled score
    pair + one AV pair (+1 slack matmul slot), staying under the EXP
    period so the EXP queue never starves.
  - Normalization is deferred off the critical path: pAB is copied to
    SBUF (uoT) and the reciprocal/broadcast/multiply chain runs behind
    the next block, with the final multiply on the Pool engine.
  - Wo matmuls of block j-1 fill the phase-2 slack slots one at a
    time; output stores go on the Sync DMA queue.

Layout notes (per core):
  qT   [128, 2, 2048]  head-pair p: head 2p on partitions 0:64, head
                       2p+1 on 64:128; RoPE applied; fp16.
  kTd  [128, 2048]     kv head duplicated on both partition halves
                       (lhsT of the two row-tiled score matmuls); fp16.
  V    [128, 16, 65]   [sk-chunk, 65] fp16; col 64 = 1.0 so the AV
                       matmul also emits softmax denominators.
  uoT/aoT [128, 2, 2048] un/normalized attention outputs, o-major,
                       fp16; aoT is lhsT of the Wo matmul.
"""

import sys

if "/opt/trn_rl_repo" not in sys.path:
    sys.path.insert(0, "/opt/trn_rl_repo")

import numpy as np

import concourse.bass as bass
from concourse import bacc, mybir
from concourse.bass import ts
from concourse.bass_utils import run_bass_kernel_spmd
from concourse.tile import TileContext

F32 = mybir.dt.float32
F16 = mybir.dt.float16

S = 2048
H = 2048
N_HEADS = 32
N_KV = 8
D = 64
NCORES = 8
HPC = N_HEADS // NCORES  # 4 q heads per core
OC = HPC * D  # 256 q dims per core
NB = S // 512  # 4 s-blocks of 512
HC = H // 128  # 16 hidden chunks

LAST_EXEC_NS = None
LAST_TRACE = None
LAST_RES = None
_CACHE = {}


def _ternarize(w):
    w = np.asarray(w, np.float32)
    s = (np.abs(w).mean() + np.float32(1e-6)).astype(np.float32)
    t = np.round(np.clip(w / s, np.float32(-1.0), np.float32(1.0))).astype(np.float32)
    return t, float(s)


def _build_program(s_qk):
    nc = bacc.Bacc("TRN2", target_bir_lowering=False, debug=False, num_devices=NCORES)

    xt = nc.dram_tensor("xt", [NB, 128, HC, 512], F16, kind="ExternalInput")
    wq = nc.dram_tensor("wq_t", [128, HC, OC], F16, kind="ExternalInput")
    wkv = nc.dram_tensor("wkv_t", [128, HC, 128], F16, kind="ExternalInput")
    wo = nc.dram_tensor("wo_t", [128, 2, H], F16, kind="ExternalInput")
    cos_d = nc.dram_tensor("cos_t", [64, S], F16, kind="ExternalInput")
    sin_d = nc.dram_tensor("sin_t", [64, S], F16, kind="ExternalInput")
    mask_d = nc.dram_tensor("mask_t", [128, HC], F32, kind="ExternalInput")
    prot_d = nc.dram_tensor("prot_t", [128, 128], F16, kind="ExternalInput")
    outp = nc.dram_tensor("outp", [S, H], F32, kind="ExternalOutput")

    EXP = mybir.ActivationFunctionType.Exp
    MUL = mybir.AluOpType.mult
    ADD = mybir.AluOpType.add

    with TileContext(nc) as tc:
        with tc.tile_pool(name="persist", bufs=1) as persist:
            qT = persist.tile([128, 2, S], F16)
            kTd = persist.tile([128, S], F16)
            V = persist.tile([128, HC, 65], F16)
            uoT = persist.tile([128, 2, S], F16)
            aoT = persist.tile([128, 2, S], F16)
            xt_sb = persist.tile([128, NB, HC, 512], F16)
            wq_sb = persist.tile([128, HC, OC], F16)
            wkv_sb = persist.tile([128, HC, 128], F16)
            wo_sb = persist.tile([128, 2, H], F16)
            cos_sb = persist.tile([128, S], F16)
            sin_sb = persist.tile([128, S], F16)
            mask_sb = persist.tile([128, HC], F32)
            prot = persist.tile([128, 128], F16)

            nc.gpsimd.memset(V[:, :, 64:65], 1.0)
            # All input loads striped across the sync+scalar queues in
            # global priority order, so the first-needed bytes (wkv, x
            # block 0, wq) win the shared HBM bandwidth.
            loads = [(wkv_sb[:], wkv[:])]
            loads += [
                (xt_sb[:, 0, ts(c4, 4), :], xt[0, :, ts(c4, 4), :])
                for c4 in range(4)
            ]
            loads += [
                (wq_sb[:, ts(c, 8), :], wq[:, ts(c, 8), :]) for c in range(2)
            ]
            loads += [
                (xt_sb[:, 1, ts(c4, 4), :], xt[1, :, ts(c4, 4), :])
                for c4 in range(4)
            ]
            loads += [
                (cos_sb[0:64, :], cos_d[:]),
                (sin_sb[0:64, :], sin_d[:]),
                (prot[:], prot_d[:]),
                (mask_sb[:], mask_d[:]),
            ]
            loads += [
                (xt_sb[:, 2, ts(c4, 4), :], xt[2, :, ts(c4, 4), :])
                for c4 in range(4)
            ]
            loads += [(wo_sb[:], wo[:])]
            loads += [
                (xt_sb[:, 3, ts(c4, 4), :], xt[3, :, ts(c4, 4), :])
                for c4 in range(4)
            ]
            for n, (dst, src) in enumerate(loads):
                (nc.sync if n % 2 == 0 else nc.scalar).dma_start(dst, src)
            # duplicate rope tables onto partitions 64:128 on-device
            nc.gpsimd.tensor_copy(cos_sb[64:128, :], cos_sb[0:64, :])
            nc.gpsimd.tensor_copy(sin_sb[64:128, :], sin_sb[0:64, :])

            with (
                tc.tile_pool(name="ph1", bufs=2) as ph1,
                tc.tile_pool(name="expp", bufs=6) as expp,
                tc.tile_pool(name="ph2t", bufs=3) as ph2t,
                tc.tile_pool(name="csd", bufs=4, space="DRAM") as csd,
                tc.tile_pool(name="osp", bufs=4) as osp,
                tc.tile_pool(name="pssc", bufs=2, space="PSUM") as pssc,
                tc.tile_pool(name="psav", bufs=1, space="PSUM") as psav,
                tc.tile_pool(name="pso", bufs=2, space="PSUM") as pso_,
            ):
                # ---------- phase-1 emission helpers ----------
                def emit_kv(b):
                    pkv = pssc.tile([128, 1024], F32, tag="sAB")
                    for c in range(HC):
                        nc.tensor.matmul(
                            pkv[:, 0:512], wkv_sb[:, c, :], xt_sb[:, b, c, :],
                            start=c == 0, stop=c == HC - 1,
                            skip_group_check=True,
                        )
                    ksb = ph1.tile([64, 512], F16, tag="ksb")
                    nc.scalar.copy(ksb[:], pkv[0:64, 0:512])
                    vt = ph1.tile([64, 512], F16, tag="vt")
                    nc.scalar.copy(vt[:], pkv[64:128, 0:512])
                    return ksb, vt

                def emit_kv_tail(b, ksb, vt):
                    sb = ts(b, 512)
                    for i4 in range(4):
                        vtt = ph1.tile([128, 64], F16, tag=f"vtt{i4 % 2}")
                        nc.sync.dma_start_transpose(vtt[:], vt[:, ts(i4, 128)])
                        nc.gpsimd.tensor_copy(V[:, 4 * b + i4, 0:64], vtt[:])
                    rotk = pso_.tile([128, 512], F32, tag="po")
                    nc.tensor.matmul(
                        rotk[0:64, :], prot[0:64, 0:64], ksb[:],
                        start=True, stop=True,
                    )
                    kc = ph1.tile([64, 512], F16, tag="kc")
                    nc.gpsimd.tensor_tensor(kc[:], ksb[:], cos_sb[0:64, sb], MUL)
                    rks = ph1.tile([64, 512], F16, tag="rks")
                    nc.scalar.copy(rks[:], rotk[0:64, :])
                    ks = ph1.tile([64, 512], F16, tag="ks")
                    nc.vector.tensor_tensor(ks[:], rks[:], sin_sb[0:64, sb], MUL)
                    nc.vector.tensor_tensor(kTd[0:64, sb], kc[:], ks[:], ADD)
                    nc.gpsimd.tensor_copy(kTd[64:128, sb], kTd[0:64, sb])

                def emit_q(b):
                    pq = pssc.tile([128, 1024], F32, tag="sAB")
                    for c in range(HC):
                        nc.tensor.matmul(
                            pq[:, 0:512], wq_sb[:, c, 0:128], xt_sb[:, b, c, :],
                            start=c == 0, stop=c == HC - 1,
                            skip_group_check=True,
                        )
                        nc.tensor.matmul(
                            pq[:, 512:1024], wq_sb[:, c, 128:256],
                            xt_sb[:, b, c, :],
                            start=c == 0, stop=c == HC - 1,
                            skip_group_check=True,
                        )
                    qsb0 = ph1.tile([128, 512], F16, tag="qsb0")
                    nc.scalar.copy(qsb0[:], pq[:, 0:512])
                    qsb1 = ph1.tile([128, 512], F16, tag="qsb1")
                    nc.scalar.copy(qsb1[:], pq[:, 512:1024])
                    return qsb0, qsb1

                def emit_q_rot(b, p, qsb):
                    rotq = pso_.tile([128, 512], F32, tag="po")
                    nc.tensor.matmul(
                        rotq[:], prot[:], qsb[:], start=True, stop=True
                    )
                    return rotq

                def emit_q_tail(b, p, qsb, rotq):
                    sb = ts(b, 512)
                    qc = ph1.tile([128, 512], F16, tag=f"qc{p}")
                    nc.gpsimd.tensor_tensor(qc[:], qsb[:], cos_sb[:, sb], MUL)
                    rqs = ph1.tile([128, 512], F16, tag=f"rqs{p}")
                    nc.scalar.copy(rqs[:], rotq[:])
                    qs = ph1.tile([128, 512], F16, tag=f"qs{p}")
                    nc.vector.tensor_tensor(qs[:], rqs[:], sin_sb[:, sb], MUL)
                    nc.vector.tensor_tensor(qT[:, p, sb], qc[:], qs[:], ADD)

                # ---------- phase 1: blocks 0..2 fully, block 3 kv only ----
                # PE order: kv(b), q(b), rotk(b), rotq(b-1,*) — every rot
                # matmul waits a ScalarE copy that ran during the previous
                # projection, so the PE never stalls.
                prev_q = None
                for b in range(3):
                    ksb, vt = emit_kv(b)
                    qsb0, qsb1 = emit_q(b)
                    emit_kv_tail(b, ksb, vt)
                    if prev_q is not None:
                        pb, p0, p1 = prev_q
                        r0 = emit_q_rot(pb, 0, p0)
                        r1 = emit_q_rot(pb, 1, p1)
                        emit_q_tail(pb, 0, p0, r0)
                        emit_q_tail(pb, 1, p1, r1)
                    prev_q = (b, qsb0, qsb1)
                ksb3, vt3 = emit_kv(3)
                pb, p0, p1 = prev_q
                r0 = emit_q_rot(pb, 0, p0)
                r1 = emit_q_rot(pb, 1, p1)
                emit_q_tail(pb, 0, p0, r0)
                emit_q_tail(pb, 1, p1, r1)
                emit_kv_tail(3, ksb3, vt3)

                # ---------- phase-2 slack slots ----------
                # j=0 carries block 3's q projection; j>=1 carries Wo(j-1).
                slots = []

                def queue_q3():
                    pq = pssc.tile([128, 1024], F32, tag="sAB")
                    for c in range(HC):
                        def mm(c=c, half=0):
                            nc.tensor.matmul(
                                pq[:, ts(half, 512)],
                                wq_sb[:, c, ts(half, 128)],
                                xt_sb[:, 3, c, :],
                                start=c == 0, stop=c == HC - 1,
                                skip_group_check=True,
                            )
                        slots.append(lambda c=c: mm(c, 0))
                        slots.append(lambda c=c: mm(c, 1))

                    def cp():
                        qsb0 = ph1.tile([128, 512], F16, tag="qsb0")
                        nc.vector.tensor_copy(qsb0[:], pq[:, 0:512])
                        qsb1 = ph1.tile([128, 512], F16, tag="qsb1")
                        nc.vector.tensor_copy(qsb1[:], pq[:, 512:1024])
                        qs3.extend([qsb0, qsb1])
                    slots.append(cp)
                    for p in range(2):
                        def rot_and_tail(p=p):
                            sb = ts(3, 512)
                            qsb = qs3[p]
                            rotq = pso_.tile([128, 512], F32, tag="po")
                            nc.tensor.matmul(
                                rotq[:], prot[:], qsb[:], start=True, stop=True
                            )
                            qc = ph1.tile([128, 512], F16, tag=f"qc{p}")
                            nc.gpsimd.tensor_tensor(
                                qc[:], qsb[:], cos_sb[:, sb], MUL
                            )
                            rqs = ph1.tile([128, 512], F16, tag=f"rqs{p}")
                            nc.vector.tensor_copy(rqs[:], rotq[:])
                            qs = ph1.tile([128, 512], F16, tag=f"qs{p}")
                            nc.vector.tensor_tensor(
                                qs[:], rqs[:], sin_sb[:, sb], MUL
                            )
                            nc.vector.tensor_tensor(
                                qT[:, p, sb], qc[:], qs[:], ADD
                            )
                        slots.append(rot_and_tail)

                qs3 = []

                def queue_wo(j, tail=False):
                    for jq4 in range(4):
                        jq = 4 * j + jq4
                        for hb in range(4):
                            po = pso_.tile(
                                [128, 512], F32, tag="po", name=f"po_{jq}_{hb}"
                            )
                            # in the tail drain, ScalarE is idle: split the
                            # PSUM->SBUF copies between it and the DVE
                            cp_eng = (
                                nc.scalar if (tail and hb % 2 == 0) else None
                            )

                            def mm0(po=po, jq=jq, hb=hb):
                                nc.tensor.matmul(
                                    po[:], aoT[:, 0, ts(jq, 128)],
                                    wo_sb[:, 0, ts(hb, 512)],
                                    start=True, stop=False,
                                    skip_group_check=True,
                                )

                            def mm1(po=po, jq=jq, hb=hb, cp_eng=cp_eng):
                                nc.tensor.matmul(
                                    po[:], aoT[:, 1, ts(jq, 128)],
                                    wo_sb[:, 1, ts(hb, 512)],
                                    start=False, stop=True,
                                    skip_group_check=True,
                                )
                                ob = osp.tile(
                                    [128, 512], F32, tag="ob", name=f"ob{jq}_{hb}"
                                )
                                if cp_eng is not None:
                                    cp_eng.copy(ob[:], po[:])
                                else:
                                    nc.vector.tensor_copy(ob[:], po[:])
                                nc.sync.dma_start(
                                    outp[ts(jq, 128), ts(hb, 512)], ob[:]
                                )

                            slots.append(mm0)
                            slots.append(mm1)

                def emit_slot():
                    if slots:
                        slots.pop(0)()

                # ---------- phase 2: flat 128-chunk pipeline ----------
                # Scores+EXP stream continuously; AV lags 5 chunks so a
                # block's last AVs drain during the next block's chunks and
                # the EXP queue never pauses at block boundaries.
                queue_q3()
                LAG = 5
                chunk = [
                    (j, p, i)
                    for j in range(NB)
                    for p in range(2)
                    for i in range(HC)
                ]
                state = {}

                def emit_norm(j, p):
                    jb = ts(j, 512)
                    pAB = state.pop((j, p))["pAB"]
                    nc.vector.tensor_copy(uoT[0:64, p, jb], pAB[0:64, 0:512])
                    nc.vector.tensor_copy(
                        uoT[64:128, p, jb], pAB[0:64, 512:1024]
                    )
                    cs = ph2t.tile([33, 512], F32, tag="cs")
                    nc.vector.tensor_copy(cs[0:1, :], pAB[64:65, 0:512])
                    nc.vector.tensor_copy(cs[32:33, :], pAB[64:65, 512:1024])
                    rcs = ph2t.tile([33, 512], F32, tag="rcs")
                    nc.vector.reciprocal(rcs[:], cs[:])
                    cs_dram = csd.tile([2, 1, 512], F32, tag="csd")
                    nc.sync.dma_start(cs_dram[0], rcs[0:1, :])
                    nc.sync.dma_start(cs_dram[1], rcs[32:33, :])
                    cb = ph2t.tile([128, 512], F32, tag="cb")
                    nc.sync.dma_start(
                        cb[0:64, :], cs_dram[0].to_broadcast((64, 512))
                    )
                    nc.sync.dma_start(
                        cb[64:128, :], cs_dram[1].to_broadcast((64, 512))
                    )
                    nc.gpsimd.tensor_tensor(
                        aoT[0:64, p, jb], uoT[0:64, p, jb], cb[0:64, :], MUL
                    )
                    nc.gpsimd.tensor_tensor(
                        aoT[64:128, p, jb], uoT[64:128, p, jb],
                        cb[64:128, :], MUL
                    )

                def emit_av_g(g):
                    j, p, i = chunk[g]
                    st = state[(j, p)]
                    e2 = st["e2s"].pop(i)
                    pAB = st["pAB"]
                    first, last = i == 0, i == HC - 1
                    nc.tensor.matmul(
                        pAB[:, 0:512], V[:, i, :], e2[:, 0:512],
                        start=first, stop=last, skip_group_check=True,
                    )
                    nc.tensor.matmul(
                        pAB[:, 512:1024], V[:, i, :], e2[:, 512:1024],
                        start=first, stop=last, skip_group_check=True,
                    )
                    if last:
                        emit_norm(j, p)

                for g in range(len(chunk)):
                    j, p, i = chunk[g]
                    jb = ts(j, 512)
                    if i == 0:
                        state[(j, p)] = {
                            "pAB": psav.tile(
                                [65, 1024], F32, tag="av", name=f"pAB_{j}_{p}"
                            ),
                            "e2s": {},
                        }
                        if p == 0 and j > 0:
                            queue_wo(j - 1)
                    psAB = pssc.tile([128, 1024], F32, tag="sAB")
                    nc.tensor.matmul(
                        psAB[:, 0:512], kTd[0:64, ts(i, 128)],
                        qT[0:64, p, jb], start=True, stop=True,
                        skip_group_check=True,
                    )
                    nc.tensor.matmul(
                        psAB[:, 512:1024], kTd[64:128, ts(i, 128)],
                        qT[64:128, p, jb], start=True, stop=True,
                        skip_group_check=True,
                    )
                    e2 = expp.tile(
                        [128, 1024], F16, tag="e2", name=f"e2_{j}_{p}_{i}"
                    )
                    nc.scalar.activation(
                        e2[:], psAB[:], EXP,
                        bias=mask_sb[:, i : i + 1], scale=s_qk,
                    )
                    state[(j, p)]["e2s"][i] = e2
                    if g >= LAG:
                        emit_av_g(g - LAG)
                    if g % 32 >= 4:
                        emit_slot()
                        if len(slots) > 32 - (g % 32):
                            emit_slot()
                for g in range(len(chunk) - LAG, len(chunk)):
                    emit_av_g(g)
                queue_wo(NB - 1, tail=True)
                while slots:
                    emit_slot()

    nc.compile()
    return nc


def kernel(
    hidden_states,
    attention_mask,
    position_ids,
    wq,
    wk,
    wv,
    wo,
    _trace=False,
):
    global LAST_EXEC_NS, LAST_TRACE, LAST_RES
    x = np.asarray(hidden_states, np.float32)[0]  # [S, H]
    mask = np.asarray(attention_mask, np.float32)[0]  # [S]
    pos = np.asarray(position_ids)[0].astype(np.float32)  # [S]

    wq_t, s_q = _ternarize(wq)
    wk_t, s_k = _ternarize(wk)
    wv_t, s_v = _ternarize(wv)
    wo_t, s_o = _ternarize(wo)
    s_qk = float(np.float32(s_q) * np.float32(s_k) / np.float32(8.0))
    s_vo = np.float32(s_v) * np.float32(s_o)

    key = ("v8", s_qk)
    if key not in _CACHE:
        _CACHE.clear()
        _CACHE[key] = _build_program(s_qk)
    nc = _CACHE[key]

    # shared inputs
    xt_host = np.ascontiguousarray(
        x.T.reshape(HC, 128, NB, 512).transpose(2, 1, 0, 3)
    ).astype(np.float16)
    inv = (
        1.0 / (10000.0 ** (np.arange(0, D, 2, dtype=np.float32) / np.float32(D)))
    ).astype(np.float32)
    fr = pos[:, None] * inv[None, :]  # [S, 32]
    emb = np.concatenate([fr, fr], axis=1)  # [S, 64]
    cos64 = np.cos(emb).astype(np.float32)
    sin64 = np.sin(emb).astype(np.float32)
    sin64[:, : D // 2] *= -1.0
    cos128 = np.ascontiguousarray(cos64.T).astype(np.float16)
    sin128 = np.ascontiguousarray(sin64.T).astype(np.float16)
    mask_r = np.ascontiguousarray(mask.reshape(HC, 128).T).astype(np.float32)
    # rotate-half block-swap permutation (lhsT), sign lives in sin128
    i32 = np.eye(32, dtype=np.float16)
    z32 = np.zeros((32, 32), dtype=np.float16)
    p64 = np.block([[z32, i32], [i32, z32]]).astype(np.float16)
    prot_host = np.zeros((128, 128), dtype=np.float16)
    prot_host[0:64, 0:64] = p64
    prot_host[64:128, 64:128] = p64

    in_maps = []
    for c in range(NCORES):
        wq_c = np.ascontiguousarray(
            wq_t[c * OC : (c + 1) * OC, :].T.reshape(HC, 128, OC).transpose(1, 0, 2)
        ).astype(np.float16)
        wk_c = wk_t[c * D : (c + 1) * D, :].T  # [H, 64]
        wv_c = (wv_t[c * D : (c + 1) * D, :] * s_vo).T  # fold s_v*s_o into wv
        wkv_c = np.ascontiguousarray(
            np.concatenate([wk_c, wv_c], axis=1).reshape(HC, 128, 128).transpose(1, 0, 2)
        ).astype(np.float16)
        wo_c = np.ascontiguousarray(
            wo_t[:, c * OC : (c + 1) * OC].T.reshape(2, 128, H).transpose(1, 0, 2)
        ).astype(np.float16)
        in_maps.append(
            {
                "xt": xt_host,
                "wq_t": wq_c,
                "wkv_t": wkv_c,
                "wo_t": wo_c,
                "cos_t": cos128,
                "sin_t": sin128,
                "mask_t": mask_r,
                "prot_t": prot_host,
            }
        )

    res = run_bass_kernel_spmd(
        nc, in_maps, core_ids=list(range(NCORES)), trace=bool(_trace)
    )
    LAST_EXEC_NS = res.exec_time_ns
    LAST_TRACE = res.instructions_and_trace[1] if res.instructions_and_trace else None
    LAST_RES = res

    out = res.results[0]["outp"].astype(np.float32)
    for c in range(1, NCORES):
        out = out + res.results[c]["outp"].astype(np.float32)
    return out.reshape(1, S, H).astype(np.float32)
